# revision 14
# baseline (speedup 1.0000x reference)
"""Matching Network (retrieval_knn) Trainium2 Bass kernel.

Data-parallel over the 32 episodes: 4 episodes (120 images) per core x 8 cores.
Per-core pipeline (fp32 storage, fp32r matmuls):
  L1 conv(1->64, 3x3 same) via host-side 9-row im2col -> maxpool2x2 -> BN+ReLU
  L2..L4 conv(64->64) as 9 accumulating shift-matmuls -> maxpool2x2 -> BN+ReLU
  (pool commutes with the per-channel monotone affine+relu, so pool runs on raw
   PSUM conv outputs and the affine+relu on the 4x smaller pooled tensor)
  cosine-attention head: sims / softmax / preds / CE / correct-count.
Host gathers: ce[32] concat, acc = sum(correct)/160.
"""

import sys

sys.path.insert(0, "/opt/trn_rl_repo")

import numpy as np

import concourse.bass as bass
import concourse.bacc as bacc
import concourse.tile as tile
from concourse import mybir

F32 = mybir.dt.float32
F32R = mybir.dt.float32r
U32 = mybir.dt.uint32
AX = mybir.AxisListType
ALU = mybir.AluOpType
ACT = mybir.ActivationFunctionType

B, NS, NQ, H, W = 32, 25, 5, 28, 28
NCLS, F = 5, 64
BN_EPS = 1e-3
NCORE = 8
EP = B // NCORE            # 4 episodes per core
NIMG = EP * (NS + NQ)      # 120 images per core
NSUP = EP * NS             # 100 support images
G = 8                      # images per pipeline group
NGRP = NIMG // G           # 15 groups


def apx(t, free_dims, offset=0, p0=0, pn=None):
    """Custom AP over tile t: partition range [p0, p0+pn) and free [step,count]
    dims (elements)."""
    row = t.ap[0][0]
    if pn is None:
        pn = t.ap[0][1]
    return bass.AP(t.tensor, t.offset + p0 * row + offset, [[row, pn]] + list(free_dims))


def build_bass():
    nc = bacc.Bacc("TRN2", target_bir_lowering=False, debug=False)

    # ---- DRAM I/O ----
    x_l1 = nc.dram_tensor("x_l1", [NGRP, 9, G * 784], F32R, kind="ExternalInput").ap()
    w1_d = nc.dram_tensor("w1", [9, F], F32R, kind="ExternalInput").ap()
    wc_d = nc.dram_tensor("wconst", [F, 3 * 576 + 1], F32R, kind="ExternalInput").ap()
    ab_d = nc.dram_tensor("wab", [F, 8], F32, kind="ExternalInput").ap()
    yh_d = nc.dram_tensor("yhot", [NS, 24], F32R, kind="ExternalInput").ap()
    yqh_d = nc.dram_tensor("yqhot", [NQ, EP * NCLS], F32, kind="ExternalInput").ap()
    fs_d = nc.dram_tensor("finscale", [8, 1], F32, kind="ExternalInput").ap()
    out_d = nc.dram_tensor("out", [1, 8], F32, kind="ExternalOutput").ap()

    with tile.TileContext(nc) as tc:
        with (
            tc.tile_pool(name="const", bufs=1) as cpool,
            tc.tile_pool(name="l1in", bufs=3) as l1pool,
            tc.tile_pool(name="t1", bufs=3) as t1pool,
            tc.tile_pool(name="pad", bufs=2) as padpool,
            tc.tile_pool(name="persist", bufs=1) as perpool,
            tc.tile_pool(name="head", bufs=1) as headpool,
            tc.tile_pool(name="psum", bufs=2, space="PSUM") as pspool,
        ):
            # ---- constants ----
            w1 = cpool.tile([9, F], F32R)
            nc.sync.dma_start(out=w1[:, :], in_=w1_d[:, :])
            wc = cpool.tile([F, 3 * 576 + 1], F32R)
            nc.sync.dma_start(out=wc[:, :], in_=wc_d[:, :])
            wab = cpool.tile([F, 8], F32)
            nc.sync.dma_start(out=wab[:, :], in_=ab_d[:, :])
            yh = cpool.tile([NS, 24], F32R)
            nc.sync.dma_start(out=yh[:, :], in_=yh_d[:, :])
            yqh = cpool.tile([NQ, EP * NCLS], F32)
            nc.sync.dma_start(out=yqh[:, :], in_=yqh_d[:, :])
            fsc = cpool.tile([8, 1], F32)
            nc.sync.dma_start(out=fsc[:, :], in_=fs_d[:, :])

            def w_l(layer, s):  # lhsT [64, 64] for conv layer 2..4, shift s
                off = (layer - 2) * 576 + s * F
                return wc[:, off : off + F]

            def ab_l(layer):  # (A, B) per-channel affine [64,1] each
                base = 2 * (layer - 1)
                return wab[:, base : base + 1], wab[:, base + 1 : base + 2]

            ones64 = wc[:, 3 * 576 : 3 * 576 + 1]

            # ---- persistent tiles ----
            p4 = perpool.tile([F, NIMG * 25], F32R)  # padded 5x5 L4 inputs
            nc.vector.memset(p4[:, :].bitcast(U32), 0)
            emb = perpool.tile([F, NIMG], F32R)

            for g in range(NGRP):
                # ---------- load L1 im2col for this group ----------
                l1t = l1pool.tile([9, G * 784], F32R)
                nc.sync.dma_start(out=l1t[:, :], in_=x_l1[g])

                # ---------- L1 conv ----------
                # p2t: 4 pair-blocks of 512; block = (y 0..15, x 0..15, i 0..1)
                p2t = padpool.tile([F, G * 256], F32R, tag="p2")
                nc.vector.memset(apx(p2t, [[512, 4], [480, 2], [1, 32]]).bitcast(U32), 0)
                nc.vector.memset(apx(p2t, [[512, 4], [32, 14], [1, 2]], offset=32).bitcast(U32), 0)
                nc.vector.memset(apx(p2t, [[512, 4], [32, 14], [1, 2]], offset=62).bitcast(U32), 0)
                a1, b1 = ab_l(1)
                for half in range(G // 2):
                    ps = pspool.tile([F, 2048], F32, tag="ps")
                    for i in range(2):       # image within pair
                        img = half * 2 + i
                        for h in range(2):   # 14-row half of 28x28 output
                            nc.tensor.matmul(
                                ps[:, 512 * (2 * i + h) : 512 * (2 * i + h) + 392],
                                w1[:, :],
                                l1t[:, img * 784 + h * 392 : img * 784 + (h + 1) * 392],
                                start=True,
                                stop=True,
                            )
                    # BN+ReLU evacuation PSUM->SBUF (hb order A1 A2 B1 B2)
                    sb1 = t1pool.tile([F, 4 * 392], F32, tag="sb")
                    nc.scalar.activation(
                        sb1[:, :], apx(ps, [[512, 4], [1, 392]]),
                        ACT.Relu, bias=b1, scale=a1,
                    )
                    # maxpool 28x28 -> 14x14 on SBUF
                    ct = t1pool.tile([F, 4 * 196], F32, tag="t1")
                    nc.vector.tensor_tensor(
                        ct[:, :],
                        apx(sb1, [[392, 4], [28, 14], [2, 14]]),
                        apx(sb1, [[392, 4], [28, 14], [2, 14]], offset=1),
                        op=ALU.max,
                    )
                    for par in range(2):  # h1 blocks -> rows 1-7, h2 -> 8-14
                        nc.vector.tensor_tensor(
                            apx(p2t, [[1, 2], [32, 7], [2, 14]],
                                offset=half * 512 + (1 + 7 * par) * 32 + 2),
                            apx(ct, [[392, 2], [28, 7], [1, 14]], offset=par * 196),
                            apx(ct, [[392, 2], [28, 7], [1, 14]], offset=par * 196 + 14),
                            op=ALU.max,
                        )

                # ---------- L2 conv ----------
                # p3t: (y 0..8, x 0..8, i 0..7), row width 72
                p3t = padpool.tile([F, G * 81], F32R, tag="p3")
                nc.vector.memset(apx(p3t, [[576, 2], [1, 72]]).bitcast(U32), 0)
                nc.vector.memset(apx(p3t, [[72, 7], [1, 8]], offset=72).bitcast(U32), 0)
                nc.vector.memset(apx(p3t, [[72, 7], [1, 8]], offset=136).bitcast(U32), 0)
                ps2 = pspool.tile([F, 2048], F32, tag="ps")
                for pr in range(4):
                    for s in range(9):
                        dy, dx = divmod(s, 3)
                        nc.tensor.matmul(
                            ps2[:, 512 * pr : 512 * pr + 392],
                            w_l(2, s),
                            apx(p2t, [[32, 14], [1, 28]],
                                offset=pr * 512 + dy * 32 + dx * 2),
                            start=(s == 0),
                            stop=(s == 8),
                        )
                # BN+ReLU evacuation -> sb2; psum block = (y 14, x 14, i 2)
                a2, b2 = ab_l(2)
                sb2 = t1pool.tile([F, 8 * 196], F32, tag="sb")
                nc.scalar.activation(
                    sb2[:, :], apx(ps2, [[512, 4], [1, 392]]),
                    ACT.Relu, bias=b2, scale=a2,
                )
                # maxpool 14x14 -> 7x7; sb2 = (pair, y, x, i) with pair*y
                # merged (contiguous): [[28, 56], [4, 7], [1, 2]]
                ct2 = t1pool.tile([F, 8 * 98], F32, tag="t1")
                nc.vector.tensor_tensor(
                    ct2[:, :],
                    apx(sb2, [[28, 56], [4, 7], [1, 2]]),
                    apx(sb2, [[28, 56], [4, 7], [1, 2]], offset=2),
                    op=ALU.max,
                )
                # rowmax into p3t interior, split by i parity
                for i in range(2):
                    nc.vector.tensor_tensor(
                        apx(p3t, [[2, 4], [72, 7], [8, 7]], offset=80 + i),
                        apx(ct2, [[196, 4], [28, 7], [2, 7]], offset=i),
                        apx(ct2, [[196, 4], [28, 7], [2, 7]], offset=14 + i),
                        op=ALU.max,
                    )

                # ---------- L3 conv: all 8 imgs, one N=392 chain ----------
                ps3 = pspool.tile([F, 2048], F32, tag="ps")
                for s in range(9):
                    dy, dx = divmod(s, 3)
                    nc.tensor.matmul(
                        ps3[:, 0:392],
                        w_l(3, s),
                        apx(p3t, [[72, 7], [1, 56]], offset=dy * 72 + dx * 8),
                        start=(s == 0),
                        stop=(s == 8),
                    )
                # BN+ReLU evacuation -> sb3; psum = (y 7, x 7, i 8)
                a3, b3 = ab_l(3)
                sb3 = t1pool.tile([F, G * 49], F32, tag="t1b")
                nc.scalar.activation(
                    sb3[:, :], ps3[:, 0:392], ACT.Relu, bias=b3, scale=a3,
                )
                # maxpool 7x7 -> 3x3 (uses y,x 0..5) into p4 interior
                ct3 = t1pool.tile([F, G * 18], F32, tag="t1c")
                nc.vector.tensor_tensor(
                    ct3[:, :],
                    apx(sb3, [[56, 6], [16, 3], [1, 8]]),
                    apx(sb3, [[56, 6], [16, 3], [1, 8]], offset=8),
                    op=ALU.max,
                )
                nc.vector.tensor_tensor(
                    apx(p4, [[200, 3], [40, 3], [1, 8]],
                        offset=(g // 5) * 1000 + 240 + (g % 5) * 8),
                    apx(ct3, [[48, 3], [1, 24]]),
                    apx(ct3, [[48, 3], [1, 24]], offset=24),
                    op=ALU.max,
                )

            # ---------- L4 conv: 3 blocks of 40 images ----------
            a4, b4 = ab_l(4)
            for blk in range(3):
                ps4 = pspool.tile([F, 2048], F32, tag="ps")
                for s in range(9):
                    dy, dx = divmod(s, 3)
                    nc.tensor.matmul(
                        ps4[:, 0:360],
                        w_l(4, s),
                        apx(p4, [[200, 3], [1, 120]],
                            offset=blk * 1000 + dy * 200 + dx * 40),
                        start=(s == 0),
                        stop=(s == 8),
                    )
                # BN+ReLU evacuation -> sb4 (y 3, x 3, i 40); 2x2-window max
                sb4 = t1pool.tile([F, 360], F32, tag="t1c")
                nc.scalar.activation(
                    sb4[:, :], ps4[:, 0:360], ACT.Relu, bias=b4, scale=a4,
                )
                nc.vector.tensor_reduce(
                    emb[:, blk * 40 : (blk + 1) * 40],
                    apx(sb4, [[1, 40], [120, 2], [40, 2]]),
                    axis=AX.XY,
                    op=ALU.max,
                )

            # ================= matching head =================
            sq = headpool.tile([F, NSUP], F32R)
            nc.scalar.square(sq[:, :], emb[:, 0:NSUP])
            ps_mag = pspool.tile([1, 512], F32, tag="ps")
            nc.tensor.matmul(
                ps_mag[0:1, 0:NSUP], ones64, sq[:, :],
                start=True, stop=True,
            )
            magc = headpool.tile([1, NSUP], F32)
            nc.vector.tensor_scalar_max(magc[:, :], ps_mag[0:1, 0:NSUP], 1e-10)
            nc.scalar.sqrt(magc[:, :], magc[:, :])
            invmag = headpool.tile([1, NSUP], F32R)
            with nc.allow_low_precision(reason="float32r keeps 19-bit mantissa"):
                nc.vector.reciprocal(invmag[:, :], magc[:, :])
            onesr = headpool.tile([1, F], F32R)
            nc.vector.memset(onesr[:, :].bitcast(U32), 0x3F800000)
            ps_bc = pspool.tile([F, 512], F32, tag="ps")
            nc.tensor.matmul(
                ps_bc[:, 0:NSUP], onesr[:, :], invmag[:, :],
                start=True, stop=True,
            )
            ssc = headpool.tile([F, NSUP], F32R)
            nc.vector.tensor_mul(ssc[:, :], emb[:, 0:NSUP], ps_bc[0:F, 0:NSUP])

            # sims^T: per episode one MM vs ALL 20 queries (N even);
            # valid cols of block e are 20e + (5e..5e+5) = 25e + q
            ps_st = pspool.tile([32, 512], F32, tag="ps")
            for e in range(EP):
                nc.tensor.matmul(
                    ps_st[0:NS, 20 * e : 20 * e + 20],
                    ssc[:, 25 * e : 25 * e + 25],
                    emb[:, NSUP : NSUP + 20],
                    start=True,
                    stop=True,
                )
            st_sb = headpool.tile([32, 32], F32)
            nc.vector.memset(st_sb[:, :], 0.0)
            nc.scalar.copy(
                st_sb[0:NS, 0:20], apx(ps_st, [[25, EP], [1, 5]], pn=NS)
            )
            sims = headpool.tile([32, 32], F32)
            nc.vector.transpose(sims[:, :], st_sb[:, :])

            # softmax (unnormalized exp; renorm happens in preds division)
            rmaxn = headpool.tile([20, 1], F32)
            nc.vector.tensor_reduce(
                rmaxn[:, :], sims[0:20, 0:NS], axis=AX.X, op=ALU.max, negate=True
            )
            ex = headpool.tile([32, 32], F32)
            nc.vector.memset(ex[:, :], 0.0)
            nc.scalar.activation(
                ex[0:20, 0:NS], sims[0:20, 0:NS], ACT.Exp, bias=rmaxn[:, :]
            )
            exT32 = headpool.tile([32, 32], F32)
            nc.vector.transpose(exT32[:, :], ex[:, :])
            exT = headpool.tile([32, 32], F32R)
            nc.scalar.copy(exT[:, :], exT32[:, :])

            # preds [5q, 4e*5c]
            ps_pr = pspool.tile([NQ, 512], F32, tag="ps")
            for e in range(EP):
                nc.tensor.matmul(
                    ps_pr[0:NQ, 6 * e : 6 * e + 6],
                    exT[0:NS, 5 * e : 5 * e + 5],
                    yh[:, 6 * e : 6 * e + 6],
                    start=True,
                    stop=True,
                )
            rs = headpool.tile([NQ, EP], F32)
            nc.vector.tensor_reduce(
                rs[:, :], apx(ps_pr, [[6, EP], [1, 5]], pn=NQ),
                axis=AX.X, op=ALU.add,
            )
            rsi = headpool.tile([NQ, EP], F32)
            nc.vector.reciprocal(rsi[:, :], rs[:, :])
            pn = headpool.tile([NQ, EP * NCLS], F32)
            nc.vector.tensor_tensor(
                pn[:, :],
                apx(ps_pr, [[6, EP], [1, 5]], pn=NQ),
                apx(rsi, [[1, EP], [0, 5]]),
                op=ALU.mult,
            )
            pc = headpool.tile([NQ, EP * NCLS], F32)
            nc.vector.tensor_scalar(
                pc[:, :], pn[:, :], 1e-7, 1.0, op0=ALU.max, op1=ALU.min
            )
            lnp = headpool.tile([NQ, EP * NCLS], F32)
            nc.scalar.activation(lnp[:, :], pc[:, :], ACT.Ln)
            ll = headpool.tile([NQ, EP * NCLS], F32)
            nc.vector.tensor_mul(ll[:, :], lnp[:, :], yqh[:, :])

            big = headpool.tile([32, 32], F32)
            nc.vector.memset(big[:, :], 0.0)
            nc.vector.tensor_reduce(
                big[0:NQ, 0:EP], apx(ll, [[5, EP], [1, 5]]),
                axis=AX.X, op=ALU.add,
            )
            rmax2 = headpool.tile([NQ, EP], F32)
            nc.vector.tensor_reduce(
                rmax2[:, :], apx(pn, [[5, EP], [1, 5]]),
                axis=AX.X, op=ALU.max,
            )
            eq = headpool.tile([NQ, EP * NCLS], F32)
            nc.vector.tensor_tensor(
                eq[:, :], pn[:, :], apx(rmax2, [[1, EP], [0, 5]]), op=ALU.is_ge
            )
            hits = headpool.tile([NQ, EP * NCLS], F32)
            nc.vector.tensor_mul(hits[:, :], eq[:, :], yqh[:, :])
            nc.vector.tensor_reduce(
                big[0:NQ, 4:5], hits[:, :], axis=AX.X, op=ALU.add
            )
            bigT = headpool.tile([32, 32], F32)
            nc.vector.transpose(bigT[:, :], big[:, :])
            red = headpool.tile([8, 1], F32)
            nc.vector.tensor_reduce(
                red[0:8, 0:1], bigT[0:8, 0:NQ], axis=AX.X, op=ALU.add
            )
            fin = headpool.tile([8, 1], F32)
            nc.vector.tensor_scalar_mul(fin[:, :], red[:, :], fsc[:, :])
            nc.sync.dma_start(out=out_d[:, :], in_=fin[:, :])

    nc.compile()
    return nc


def _prep_inputs(x_support, y_support, x_query, y_query, params):
    """Host-side shard + layout prep. Returns list of 8 in_maps."""
    xs = x_support.reshape(B, NS, H, W)
    xq = x_query.reshape(B, NQ, H, W)

    w1 = params[0][0].reshape(9, F).astype(np.float32)
    wc = np.zeros((F, 3 * 576 + 1), np.float32)
    ab = np.zeros((F, 8), np.float32)
    for li in range(4):
        k, b, g_, be, m, v = params[li]
        A = (g_ / np.sqrt(v + BN_EPS)).astype(np.float32)
        Bb = (be + (b - m) * A).astype(np.float32)
        ab[:, 2 * li] = A
        ab[:, 2 * li + 1] = Bb
        if li > 0:
            wc[:, (li - 1) * 576 : li * 576] = (
                k.reshape(9, F, F).transpose(1, 0, 2).reshape(F, 576)
            )
    wc[:, 3 * 576] = 1.0
    fs = np.zeros((8, 1), np.float32)
    fs[0:4, 0] = -1.0 / NQ
    fs[4, 0] = 1.0

    in_maps = []
    for c in range(NCORE):
        eps = slice(c * EP, (c + 1) * EP)
        imgs = np.concatenate(
            [xs[eps].reshape(NSUP, H, W), xq[eps].reshape(EP * NQ, H, W)], axis=0
        )
        pad = np.zeros((NIMG, H + 2, W + 2), np.float32)
        pad[:, 1:-1, 1:-1] = imgs
        cols = np.empty((NIMG, 9, H * W), np.float32)
        for dy in range(3):
            for dx in range(3):
                cols[:, dy * 3 + dx] = pad[:, dy : dy + H, dx : dx + W].reshape(
                    NIMG, -1
                )
        x_l1 = (
            cols.reshape(NGRP, G, 9, 784)
            .transpose(0, 2, 1, 3)
            .reshape(NGRP, 9, G * 784)
            .copy()
        )
        ys = np.asarray(y_support[eps])
        yq = np.asarray(y_query[eps])
        yhot = np.zeros((NS, 24), np.float32)
        for e in range(EP):
            yhot[np.arange(NS), 6 * e + ys[e]] = 1.0
        yqhot = np.zeros((NQ, EP * NCLS), np.float32)
        for e in range(EP):
            yqhot[np.arange(NQ), 5 * e + yq[e]] = 1.0
        in_maps.append(
            {
                "x_l1": x_l1,
                "w1": w1,
                "wconst": wc,
                "wab": ab,
                "yhot": yhot,
                "yqhot": yqhot,
                "finscale": fs,
            }
        )
    return in_maps


_CACHED = {}


def kernel(x_support, y_support, x_query, y_query,
           k1, b1, g1, be1, m1, v1,
           k2, b2, g2, be2, m2, v2,
           k3, b3, g3, be3, m3, v3,
           k4, b4, g4, be4, m4, v4):
    params = [(k1, b1, g1, be1, m1, v1), (k2, b2, g2, be2, m2, v2),
              (k3, b3, g3, be3, m3, v3), (k4, b4, g4, be4, m4, v4)]
    params = [tuple(np.asarray(t, np.float32) for t in p) for p in params]
    in_maps = _prep_inputs(
        np.asarray(x_support, np.float32), np.asarray(y_support),
        np.asarray(x_query, np.float32), np.asarray(y_query), params
    )

    if "nc" not in _CACHED:
        _CACHED["nc"] = build_bass()
    nc = _CACHED["nc"]

    from concourse.bass_utils import run_bass_kernel_spmd

    res = run_bass_kernel_spmd(nc, in_maps, list(range(NCORE)))
    outs = [np.asarray(res.results[c]["out"]).reshape(8) for c in range(NCORE)]
    ce = np.concatenate([o[0:EP] for o in outs]).astype(np.float32)
    acc = np.asarray(sum(float(o[4]) for o in outs) / (B * NQ), np.float32)
    return ce, acc


# revision 15
# speedup vs baseline: 1.0632x; 1.0632x over previous
"""Matching Network (retrieval_knn) Trainium2 Bass kernel.

Data-parallel over the 32 episodes: 4 episodes (120 images) per core x 8 cores.
Per-core pipeline (fp32 storage, fp32r matmuls):
  L1 conv(1->64, 3x3 same) via host-side 9-row im2col -> maxpool2x2 -> BN+ReLU
  L2..L4 conv(64->64) as 9 accumulating shift-matmuls -> maxpool2x2 -> BN+ReLU
  (pool commutes with the per-channel monotone affine+relu, so pool runs on raw
   PSUM conv outputs and the affine+relu on the 4x smaller pooled tensor)
  cosine-attention head: sims / softmax / preds / CE / correct-count.
Host gathers: ce[32] concat, acc = sum(correct)/160.
"""

import sys

sys.path.insert(0, "/opt/trn_rl_repo")

import numpy as np

import concourse.bass as bass
import concourse.bacc as bacc
import concourse.tile as tile
from concourse import mybir

F32 = mybir.dt.float32
F32R = mybir.dt.float32r
U32 = mybir.dt.uint32
AX = mybir.AxisListType
ALU = mybir.AluOpType
ACT = mybir.ActivationFunctionType

B, NS, NQ, H, W = 32, 25, 5, 28, 28
NCLS, F = 5, 64
BN_EPS = 1e-3
NCORE = 8
EP = B // NCORE            # 4 episodes per core
NIMG = EP * (NS + NQ)      # 120 images per core
NSUP = EP * NS             # 100 support images
G = 8                      # images per pipeline group
NGRP = NIMG // G           # 15 groups


def apx(t, free_dims, offset=0, p0=0, pn=None):
    """Custom AP over tile t: partition range [p0, p0+pn) and free [step,count]
    dims (elements)."""
    row = t.ap[0][0]
    if pn is None:
        pn = t.ap[0][1]
    return bass.AP(t.tensor, t.offset + p0 * row + offset, [[row, pn]] + list(free_dims))


def build_bass():
    nc = bacc.Bacc("TRN2", target_bir_lowering=False, debug=False)

    # ---- DRAM I/O ----
    x_l1 = nc.dram_tensor("x_l1", [NGRP, 9, G * 784], F32R, kind="ExternalInput").ap()
    w1_d = nc.dram_tensor("w1", [9, F], F32R, kind="ExternalInput").ap()
    wc_d = nc.dram_tensor("wconst", [F, 3 * 576 + 1], F32R, kind="ExternalInput").ap()
    ab_d = nc.dram_tensor("wab", [F, 8], F32, kind="ExternalInput").ap()
    yh_d = nc.dram_tensor("yhot", [NS, 24], F32R, kind="ExternalInput").ap()
    yqh_d = nc.dram_tensor("yqhot", [NQ, EP * NCLS], F32, kind="ExternalInput").ap()
    fs_d = nc.dram_tensor("finscale", [8, 1], F32, kind="ExternalInput").ap()
    out_d = nc.dram_tensor("out", [1, 8], F32, kind="ExternalOutput").ap()

    with tile.TileContext(nc) as tc:
        with (
            tc.tile_pool(name="const", bufs=1) as cpool,
            tc.tile_pool(name="l1in", bufs=3) as l1pool,
            tc.tile_pool(name="t1", bufs=3) as t1pool,
            tc.tile_pool(name="pad", bufs=2) as padpool,
            tc.tile_pool(name="persist", bufs=1) as perpool,
            tc.tile_pool(name="head", bufs=1) as headpool,
            tc.tile_pool(name="psum", bufs=4, space="PSUM") as pspool,
        ):
            # ---- constants ----
            w1 = cpool.tile([9, F], F32R)
            nc.sync.dma_start(out=w1[:, :], in_=w1_d[:, :])
            wc = cpool.tile([F, 3 * 576 + 1], F32R)
            nc.sync.dma_start(out=wc[:, :], in_=wc_d[:, :])
            wab = cpool.tile([F, 8], F32)
            nc.sync.dma_start(out=wab[:, :], in_=ab_d[:, :])
            yh = cpool.tile([NS, 24], F32R)
            nc.sync.dma_start(out=yh[:, :], in_=yh_d[:, :])
            yqh = cpool.tile([NQ, EP * NCLS], F32)
            nc.sync.dma_start(out=yqh[:, :], in_=yqh_d[:, :])
            fsc = cpool.tile([8, 1], F32)
            nc.sync.dma_start(out=fsc[:, :], in_=fs_d[:, :])

            def w_l(layer, s):  # lhsT [64, 64] for conv layer 2..4, shift s
                off = (layer - 2) * 576 + s * F
                return wc[:, off : off + F]

            def ab_l(layer):  # (A, B) per-channel affine [64,1] each
                base = 2 * (layer - 1)
                return wab[:, base : base + 1], wab[:, base + 1 : base + 2]

            ones64 = wc[:, 3 * 576 : 3 * 576 + 1]

            # ---- persistent tiles ----
            p4 = perpool.tile([F, NIMG * 25], F32R)  # padded 5x5 L4 inputs
            nc.vector.memset(p4[:, :].bitcast(U32), 0)
            emb = perpool.tile([F, NIMG], F32R)

            for g in range(NGRP):
                # ---------- load L1 im2col for this group ----------
                l1t = l1pool.tile([9, G * 784], F32R)
                nc.sync.dma_start(out=l1t[:, :], in_=x_l1[g])

                # ---------- L1 conv ----------
                # p2t: 4 pair-blocks of 512; block = (y 0..15, x 0..15, i 0..1)
                p2t = padpool.tile([F, G * 256], F32R, tag="p2")
                nc.vector.memset(apx(p2t, [[512, 4], [480, 2], [1, 32]]).bitcast(U32), 0)
                nc.vector.memset(apx(p2t, [[512, 4], [32, 14], [1, 2]], offset=32).bitcast(U32), 0)
                nc.vector.memset(apx(p2t, [[512, 4], [32, 14], [1, 2]], offset=62).bitcast(U32), 0)
                a1, b1 = ab_l(1)
                for img in range(G):
                    ps = pspool.tile([F, 1024], F32, tag="ps")
                    for h in range(2):   # 14-row half of 28x28 output
                        nc.tensor.matmul(
                            ps[:, 512 * h : 512 * h + 392],
                            w1[:, :],
                            l1t[:, img * 784 + h * 392 : img * 784 + (h + 1) * 392],
                            start=True,
                            stop=True,
                        )
                    # BN+ReLU evacuation PSUM->SBUF [half(2), row(14), col(28)]
                    sb1 = t1pool.tile([F, 2 * 392], F32, tag="sb")
                    nc.scalar.activation(
                        sb1[:, :], apx(ps, [[512, 2], [1, 392]]),
                        ACT.Relu, bias=b1, scale=a1,
                    )
                    # maxpool 28x28 -> 14x14 on SBUF
                    ct = t1pool.tile([F, 2 * 196], F32, tag="t1")
                    nc.vector.tensor_tensor(
                        ct[:, :],
                        apx(sb1, [[392, 2], [28, 14], [2, 14]]),
                        apx(sb1, [[392, 2], [28, 14], [2, 14]], offset=1),
                        op=ALU.max,
                    )
                    # rowmax into p2t interior at (y 1..14, x 1..14, i=img%2)
                    nc.vector.tensor_tensor(
                        apx(p2t, [[224, 2], [32, 7], [2, 14]],
                            offset=(img // 2) * 512 + 34 + (img % 2)),
                        apx(ct, [[196, 2], [28, 7], [1, 14]]),
                        apx(ct, [[196, 2], [28, 7], [1, 14]], offset=14),
                        op=ALU.max,
                    )

                # ---------- L2 conv ----------
                # p3t: (y 0..8, x 0..8, i 0..7), row width 72
                p3t = padpool.tile([F, G * 81], F32R, tag="p3")
                nc.vector.memset(apx(p3t, [[576, 2], [1, 72]]).bitcast(U32), 0)
                nc.vector.memset(apx(p3t, [[72, 7], [1, 8]], offset=72).bitcast(U32), 0)
                nc.vector.memset(apx(p3t, [[72, 7], [1, 8]], offset=136).bitcast(U32), 0)
                a2, b2 = ab_l(2)
                sb2 = t1pool.tile([F, 8 * 196], F32, tag="sb")
                for ph in range(2):   # pairs (2*ph, 2*ph+1)
                    ps2 = pspool.tile([F, 1024], F32, tag="ps")
                    for pq in range(2):
                        pr = 2 * ph + pq
                        for s in range(9):
                            dy, dx = divmod(s, 3)
                            nc.tensor.matmul(
                                ps2[:, 512 * pq : 512 * pq + 392],
                                w_l(2, s),
                                apx(p2t, [[32, 14], [1, 28]],
                                    offset=pr * 512 + dy * 32 + dx * 2),
                                start=(s == 0),
                                stop=(s == 8),
                            )
                    # BN+ReLU evacuation of 2 pair-chains
                    nc.scalar.activation(
                        sb2[:, ph * 784 : (ph + 1) * 784],
                        apx(ps2, [[512, 2], [1, 392]]),
                        ACT.Relu, bias=b2, scale=a2,
                    )
                # maxpool 14x14 -> 7x7; sb2 = (pair, y, x, i) with pair*y
                # merged (contiguous): [[28, 56], [4, 7], [1, 2]]
                ct2 = t1pool.tile([F, 8 * 98], F32, tag="t1")
                nc.vector.tensor_tensor(
                    ct2[:, :],
                    apx(sb2, [[28, 56], [4, 7], [1, 2]]),
                    apx(sb2, [[28, 56], [4, 7], [1, 2]], offset=2),
                    op=ALU.max,
                )
                # rowmax into p3t interior, split by i parity
                for i in range(2):
                    nc.vector.tensor_tensor(
                        apx(p3t, [[2, 4], [72, 7], [8, 7]], offset=80 + i),
                        apx(ct2, [[196, 4], [28, 7], [2, 7]], offset=i),
                        apx(ct2, [[196, 4], [28, 7], [2, 7]], offset=14 + i),
                        op=ALU.max,
                    )

                # ---------- L3 conv: all 8 imgs, one N=392 chain ----------
                ps3 = pspool.tile([F, 1024], F32, tag="ps")
                for s in range(9):
                    dy, dx = divmod(s, 3)
                    nc.tensor.matmul(
                        ps3[:, 0:392],
                        w_l(3, s),
                        apx(p3t, [[72, 7], [1, 56]], offset=dy * 72 + dx * 8),
                        start=(s == 0),
                        stop=(s == 8),
                    )
                # BN+ReLU evacuation -> sb3; psum = (y 7, x 7, i 8)
                a3, b3 = ab_l(3)
                sb3 = t1pool.tile([F, G * 49], F32, tag="t1b")
                nc.scalar.activation(
                    sb3[:, :], ps3[:, 0:392], ACT.Relu, bias=b3, scale=a3,
                )
                # maxpool 7x7 -> 3x3 (uses y,x 0..5) into p4 interior
                ct3 = t1pool.tile([F, G * 18], F32, tag="t1c")
                nc.vector.tensor_tensor(
                    ct3[:, :],
                    apx(sb3, [[56, 6], [16, 3], [1, 8]]),
                    apx(sb3, [[56, 6], [16, 3], [1, 8]], offset=8),
                    op=ALU.max,
                )
                nc.vector.tensor_tensor(
                    apx(p4, [[200, 3], [40, 3], [1, 8]],
                        offset=(g // 5) * 1000 + 240 + (g % 5) * 8),
                    apx(ct3, [[48, 3], [1, 24]]),
                    apx(ct3, [[48, 3], [1, 24]], offset=24),
                    op=ALU.max,
                )

            # ---------- L4 conv: 3 blocks of 40 images ----------
            a4, b4 = ab_l(4)
            for blk in range(3):
                ps4 = pspool.tile([F, 1024], F32, tag="ps")
                for s in range(9):
                    dy, dx = divmod(s, 3)
                    nc.tensor.matmul(
                        ps4[:, 0:360],
                        w_l(4, s),
                        apx(p4, [[200, 3], [1, 120]],
                            offset=blk * 1000 + dy * 200 + dx * 40),
                        start=(s == 0),
                        stop=(s == 8),
                    )
                # BN+ReLU evacuation -> sb4 (y 3, x 3, i 40); 2x2-window max
                sb4 = t1pool.tile([F, 360], F32, tag="t1c")
                nc.scalar.activation(
                    sb4[:, :], ps4[:, 0:360], ACT.Relu, bias=b4, scale=a4,
                )
                nc.vector.tensor_reduce(
                    emb[:, blk * 40 : (blk + 1) * 40],
                    apx(sb4, [[1, 40], [120, 2], [40, 2]]),
                    axis=AX.XY,
                    op=ALU.max,
                )

            # ================= matching head =================
            sq = headpool.tile([F, NSUP], F32R)
            nc.scalar.square(sq[:, :], emb[:, 0:NSUP])
            ps_mag = pspool.tile([1, 512], F32, tag="ps")
            nc.tensor.matmul(
                ps_mag[0:1, 0:NSUP], ones64, sq[:, :],
                start=True, stop=True,
            )
            magc = headpool.tile([1, NSUP], F32)
            nc.vector.tensor_scalar_max(magc[:, :], ps_mag[0:1, 0:NSUP], 1e-10)
            nc.scalar.sqrt(magc[:, :], magc[:, :])
            invmag = headpool.tile([1, NSUP], F32R)
            with nc.allow_low_precision(reason="float32r keeps 19-bit mantissa"):
                nc.vector.reciprocal(invmag[:, :], magc[:, :])
            onesr = headpool.tile([1, F], F32R)
            nc.vector.memset(onesr[:, :].bitcast(U32), 0x3F800000)
            ps_bc = pspool.tile([F, 512], F32, tag="ps")
            nc.tensor.matmul(
                ps_bc[:, 0:NSUP], onesr[:, :], invmag[:, :],
                start=True, stop=True,
            )
            ssc = headpool.tile([F, NSUP], F32R)
            nc.vector.tensor_mul(ssc[:, :], emb[:, 0:NSUP], ps_bc[0:F, 0:NSUP])

            # sims^T: per episode one MM vs ALL 20 queries (N even);
            # valid cols of block e are 20e + (5e..5e+5) = 25e + q
            ps_st = pspool.tile([32, 512], F32, tag="ps")
            for e in range(EP):
                nc.tensor.matmul(
                    ps_st[0:NS, 20 * e : 20 * e + 20],
                    ssc[:, 25 * e : 25 * e + 25],
                    emb[:, NSUP : NSUP + 20],
                    start=True,
                    stop=True,
                )
            st_sb = headpool.tile([32, 32], F32)
            nc.vector.memset(st_sb[:, :], 0.0)
            nc.scalar.copy(
                st_sb[0:NS, 0:20], apx(ps_st, [[25, EP], [1, 5]], pn=NS)
            )
            sims = headpool.tile([32, 32], F32)
            nc.vector.transpose(sims[:, :], st_sb[:, :])

            # softmax (unnormalized exp; renorm happens in preds division)
            rmaxn = headpool.tile([20, 1], F32)
            nc.vector.tensor_reduce(
                rmaxn[:, :], sims[0:20, 0:NS], axis=AX.X, op=ALU.max, negate=True
            )
            ex = headpool.tile([32, 32], F32)
            nc.vector.memset(ex[:, :], 0.0)
            nc.scalar.activation(
                ex[0:20, 0:NS], sims[0:20, 0:NS], ACT.Exp, bias=rmaxn[:, :]
            )
            exT32 = headpool.tile([32, 32], F32)
            nc.vector.transpose(exT32[:, :], ex[:, :])
            exT = headpool.tile([32, 32], F32R)
            nc.scalar.copy(exT[:, :], exT32[:, :])

            # preds [5q, 4e*5c]
            ps_pr = pspool.tile([NQ, 512], F32, tag="ps")
            for e in range(EP):
                nc.tensor.matmul(
                    ps_pr[0:NQ, 6 * e : 6 * e + 6],
                    exT[0:NS, 5 * e : 5 * e + 5],
                    yh[:, 6 * e : 6 * e + 6],
                    start=True,
                    stop=True,
                )
            rs = headpool.tile([NQ, EP], F32)
            nc.vector.tensor_reduce(
                rs[:, :], apx(ps_pr, [[6, EP], [1, 5]], pn=NQ),
                axis=AX.X, op=ALU.add,
            )
            rsi = headpool.tile([NQ, EP], F32)
            nc.vector.reciprocal(rsi[:, :], rs[:, :])
            pn = headpool.tile([NQ, EP * NCLS], F32)
            nc.vector.tensor_tensor(
                pn[:, :],
                apx(ps_pr, [[6, EP], [1, 5]], pn=NQ),
                apx(rsi, [[1, EP], [0, 5]]),
                op=ALU.mult,
            )
            pc = headpool.tile([NQ, EP * NCLS], F32)
            nc.vector.tensor_scalar(
                pc[:, :], pn[:, :], 1e-7, 1.0, op0=ALU.max, op1=ALU.min
            )
            lnp = headpool.tile([NQ, EP * NCLS], F32)
            nc.scalar.activation(lnp[:, :], pc[:, :], ACT.Ln)
            ll = headpool.tile([NQ, EP * NCLS], F32)
            nc.vector.tensor_mul(ll[:, :], lnp[:, :], yqh[:, :])

            big = headpool.tile([32, 32], F32)
            nc.vector.memset(big[:, :], 0.0)
            nc.vector.tensor_reduce(
                big[0:NQ, 0:EP], apx(ll, [[5, EP], [1, 5]]),
                axis=AX.X, op=ALU.add,
            )
            rmax2 = headpool.tile([NQ, EP], F32)
            nc.vector.tensor_reduce(
                rmax2[:, :], apx(pn, [[5, EP], [1, 5]]),
                axis=AX.X, op=ALU.max,
            )
            eq = headpool.tile([NQ, EP * NCLS], F32)
            nc.vector.tensor_tensor(
                eq[:, :], pn[:, :], apx(rmax2, [[1, EP], [0, 5]]), op=ALU.is_ge
            )
            hits = headpool.tile([NQ, EP * NCLS], F32)
            nc.vector.tensor_mul(hits[:, :], eq[:, :], yqh[:, :])
            nc.vector.tensor_reduce(
                big[0:NQ, 4:5], hits[:, :], axis=AX.X, op=ALU.add
            )
            bigT = headpool.tile([32, 32], F32)
            nc.vector.transpose(bigT[:, :], big[:, :])
            red = headpool.tile([8, 1], F32)
            nc.vector.tensor_reduce(
                red[0:8, 0:1], bigT[0:8, 0:NQ], axis=AX.X, op=ALU.add
            )
            fin = headpool.tile([8, 1], F32)
            nc.vector.tensor_scalar_mul(fin[:, :], red[:, :], fsc[:, :])
            nc.sync.dma_start(out=out_d[:, :], in_=fin[:, :])

    nc.compile()
    return nc


def _prep_inputs(x_support, y_support, x_query, y_query, params):
    """Host-side shard + layout prep. Returns list of 8 in_maps."""
    xs = x_support.reshape(B, NS, H, W)
    xq = x_query.reshape(B, NQ, H, W)

    w1 = params[0][0].reshape(9, F).astype(np.float32)
    wc = np.zeros((F, 3 * 576 + 1), np.float32)
    ab = np.zeros((F, 8), np.float32)
    for li in range(4):
        k, b, g_, be, m, v = params[li]
        A = (g_ / np.sqrt(v + BN_EPS)).astype(np.float32)
        Bb = (be + (b - m) * A).astype(np.float32)
        ab[:, 2 * li] = A
        ab[:, 2 * li + 1] = Bb
        if li > 0:
            wc[:, (li - 1) * 576 : li * 576] = (
                k.reshape(9, F, F).transpose(1, 0, 2).reshape(F, 576)
            )
    wc[:, 3 * 576] = 1.0
    fs = np.zeros((8, 1), np.float32)
    fs[0:4, 0] = -1.0 / NQ
    fs[4, 0] = 1.0

    in_maps = []
    for c in range(NCORE):
        eps = slice(c * EP, (c + 1) * EP)
        imgs = np.concatenate(
            [xs[eps].reshape(NSUP, H, W), xq[eps].reshape(EP * NQ, H, W)], axis=0
        )
        pad = np.zeros((NIMG, H + 2, W + 2), np.float32)
        pad[:, 1:-1, 1:-1] = imgs
        cols = np.empty((NIMG, 9, H * W), np.float32)
        for dy in range(3):
            for dx in range(3):
                cols[:, dy * 3 + dx] = pad[:, dy : dy + H, dx : dx + W].reshape(
                    NIMG, -1
                )
        x_l1 = (
            cols.reshape(NGRP, G, 9, 784)
            .transpose(0, 2, 1, 3)
            .reshape(NGRP, 9, G * 784)
            .copy()
        )
        ys = np.asarray(y_support[eps])
        yq = np.asarray(y_query[eps])
        yhot = np.zeros((NS, 24), np.float32)
        for e in range(EP):
            yhot[np.arange(NS), 6 * e + ys[e]] = 1.0
        yqhot = np.zeros((NQ, EP * NCLS), np.float32)
        for e in range(EP):
            yqhot[np.arange(NQ), 5 * e + yq[e]] = 1.0
        in_maps.append(
            {
                "x_l1": x_l1,
                "w1": w1,
                "wconst": wc,
                "wab": ab,
                "yhot": yhot,
                "yqhot": yqhot,
                "finscale": fs,
            }
        )
    return in_maps


_CACHED = {}


def kernel(x_support, y_support, x_query, y_query,
           k1, b1, g1, be1, m1, v1,
           k2, b2, g2, be2, m2, v2,
           k3, b3, g3, be3, m3, v3,
           k4, b4, g4, be4, m4, v4):
    params = [(k1, b1, g1, be1, m1, v1), (k2, b2, g2, be2, m2, v2),
              (k3, b3, g3, be3, m3, v3), (k4, b4, g4, be4, m4, v4)]
    params = [tuple(np.asarray(t, np.float32) for t in p) for p in params]
    in_maps = _prep_inputs(
        np.asarray(x_support, np.float32), np.asarray(y_support),
        np.asarray(x_query, np.float32), np.asarray(y_query), params
    )

    if "nc" not in _CACHED:
        _CACHED["nc"] = build_bass()
    nc = _CACHED["nc"]

    from concourse.bass_utils import run_bass_kernel_spmd

    res = run_bass_kernel_spmd(nc, in_maps, list(range(NCORE)))
    outs = [np.asarray(res.results[c]["out"]).reshape(8) for c in range(NCORE)]
    ce = np.concatenate([o[0:EP] for o in outs]).astype(np.float32)
    acc = np.asarray(sum(float(o[4]) for o in outs) / (B * NQ), np.float32)
    return ce, acc
